# revision 27
# baseline (speedup 1.0000x reference)
"""Trainium2 Bass kernel for nn_AttentionLayer (B=4, S=4096, D=128, fp32).

Sharding: batch (4) x query-half (2) across 8 NeuronCores; the query half is
realized by a host-side column ROTATION of x^T (keys are permutation
invariant under softmax+sum), so every core runs the identical SPMD program
with its queries at columns 0..sq-1.

Structure (per core):
  scores[t,s] = x_s^T (Wq^T Wk) x_t: host precomputes gT = Wk^T Wq (fp64);
    device: GX = gT^T @ x^T once, then scores chunks = GX-chunk^T @ x^T
    (f32r, 1 cyc/col). No Q/K projections or copies exist at all.
  bq folds into the exp bias alpha[t] = SCALE*bq.k_t (extra V-proj column);
    bk cancels in softmax; bv is applied on host; a global shift C in the
    exp cancels in softmax (numerics centering).
  exp -> bf16 tiles. A slice of chunks runs as int16-Schraudolph on DVE
    (one tensor_scalar into the bf16 bit pattern) concurrently with ACT
    exps so the PE is never exp-starved (p-state stays at 2.4 GHz).
  AV: per-chunk bf16 matmuls (V in bf16: quantization error ~0.4%).
  denominator: DVE quad-sums exp chunks (bf16 4x mode, 0.25 cyc/col),
    then one replicated ones-matmul per quad accumulates in PSUM - the
    partition reduction only the PE can do, at 1/4 the matmul volume.
  num/den ship to host; host does num/den + bv in fp64 (free).

Measured HW facts this design is built on: every matmul costs ~1 cycle
per OUTPUT column at 2.4GHz (fp8 DoubleRow included, so fp8 buys nothing
once V needs hi+lo); ACT exp [128,1024] = 1.07us; DVE tensor_scalar from
PSUM = 1.19us; GPSIMD cannot touch PSUM and its casts are ~3.5us (but it
can initiate casting DMAs); PE de-ramps to 1.2GHz whenever it idles.
"""

import sys

import numpy as np

for _p in ("/opt/trn_rl_repo", "/opt/pypackages"):
    if _p not in sys.path:
        sys.path.append(_p)

B, S, D = 4, 4096, 128
N_CORES = 8
SQ = S // 2            # queries per core
SCALE = 1.0 / float(np.sqrt(D))
CSHIFT = 1.5           # global exp shift: exp(y-C); cancels in softmax
# Schraudolph (bf16 bit pattern): i16 = y*184.6635 + 16256.5 + delta
SCH_A = 128.0 / float(np.log(2.0))
SCH_DELTA = -7.0       # centers the 2^frac linear-interp overestimate


def default_exp_sched(n_pass, tch, n_dve=0):
    """Per (pass, chunk) exp engine: 'act' or 'dve' (Schraudolph).
    n_dve: int or per-pass list = offloaded chunks per pass. Offloaded
    chunks are never adjacent (DVE serializes; ACT must run concurrently)
    and sit late in pass 0 (V-prep owns the DVE early on)."""
    if isinstance(n_dve, int):
        n_dve = [n_dve] * n_pass
    sched = {}
    npair = tch // 2
    for p in range(n_pass):
        nd = min(n_dve[p] if p < len(n_dve) else n_dve[-1], npair)
        if p == 0:
            gset = set(range(npair - 2, max(-1, npair - 2 - nd), -1))
        else:
            gset = set(range(1, min(npair, 1 + nd)))
        for c in range(tch):
            g = c // 2
            sched[(p, c)] = "dve" if (g in gset and c % 2 == 1) else "act"
    return sched


def build_attention_bass(s=S, sq=SQ, sw=1024, n_dve_exp=(6, 12)):
    """Single-core SPMD program. s: keys; sq: queries; sw: pass width."""
    import concourse.bass as bass
    import concourse.mybir as mybir
    import concourse.tile as tile
    from concourse import bacc
    from contextlib import ExitStack

    f32 = mybir.dt.float32
    f32r = mybir.dt.float32r
    bf16 = mybir.dt.bfloat16
    i16 = mybir.dt.int16
    FT = mybir.ActivationFunctionType
    ALU = mybir.AluOpType

    tch = s // 128          # key chunks (128 keys each)
    n_pass = sq // sw
    nw = min(512, sw)       # matmul N width (f32r needs >=256)
    jn = sw // nw
    gxw = min(512, s)       # GX matmul chunk width
    qd = 4 if tch % 4 == 0 else 2   # chunks per denominator quad-sum
    sched = default_exp_sched(n_pass, tch, n_dve_exp)

    nc = bacc.Bacc("TRN2", target_bir_lowering=False, debug=False)

    xT = nc.dram_tensor("xT", [D, s], f32r, kind="ExternalInput").ap()
    gT = nc.dram_tensor("gT", [D, D], f32r, kind="ExternalInput").ap()
    wvT = nc.dram_tensor("wvT", [D, D + 2], f32r, kind="ExternalInput").ap()
    num_d = nc.dram_tensor("num", [D, sq], f32, kind="ExternalOutput").ap()
    den_d = nc.dram_tensor("den", [1, sq], f32, kind="ExternalOutput").ap()

    with tile.TileContext(nc) as tc, ExitStack() as ctx:
        const = ctx.enter_context(tc.tile_pool(name="const", bufs=1))
        big = ctx.enter_context(tc.tile_pool(name="big", bufs=1))
        exp_pool = ctx.enter_context(tc.tile_pool(name="exp", bufs=7))
        qs_pool = ctx.enter_context(tc.tile_pool(name="qs", bufs=2))
        stage = ctx.enter_context(tc.tile_pool(name="stage", bufs=2))

        gT_sb = const.tile([D, D], f32r, tag="gT")
        wv_sb = const.tile([D, D + 2], f32r, tag="wv")
        ones16 = const.tile([128, 128], bf16, tag="ones16")
        alpha_sb = const.tile([128, tch], f32, tag="alpha")    # alpha - C
        alpha16 = const.tile([128, tch], f32, tag="alpha16")   # schraudolph

        xT_sb = big.tile([D, s], f32r, tag="xT")
        gx_sb = big.tile([D, s], f32r, tag="gx")
        v16 = big.tile([128, s], bf16, tag="v16")    # V in [t, e] layout
        vstage = big.tile([128, s], f32, tag="vstage")

        # ---- input DMAs: many small transfers engage more of the 16 DMA
        # engines; initiation spread over three queue engines; ascending
        # order so early columns (first GX/V chunks, pass-0 queries) land
        # first.
        nc.sync.dma_start(gT_sb[:], gT)
        nc.sync.dma_start(wv_sb[:], wvT)
        xw = min(256, s)
        qengs = [nc.sync, nc.scalar, nc.gpsimd]
        for i, st in enumerate(range(0, s, xw)):
            w = min(xw, s - st)
            qengs[i % 3].dma_start(xT_sb[:, st:st + w], xT[:, st:st + w])
        nc.vector.memset(ones16[:], 1.0)

        # ---- phase A: GX projection + V/alpha; PSUM pools closed after
        qkv_ctx = ExitStack()
        gxps = qkv_ctx.enter_context(tc.tile_pool(name="gxps", bufs=3,
                                                  space="PSUM"))
        vps = qkv_ctx.enter_context(tc.tile_pool(name="vps", bufs=3,
                                                 space="PSUM"))

        def emit_gx(j):
            st, w = j * gxw, min(gxw, s - j * gxw)
            gp = gxps.tile([128, gxw], f32, tag="gx")
            nc.tensor.matmul(gp[:, :w], gT_sb[:], xT_sb[:, st:st + w])
            # ACT is idle before the first exp; DVE stays free for V-prep
            nc.scalar.copy(gx_sb[:, st:st + w], gp[:, :w])

        # V-prep: per chunk a PSUM->SBUF f32 stage copy + alpha extract on
        # DVE; the f32->bf16 cast is a bulk casting DMA per group of 8.
        vgrp = min(8, tch)

        def emit_v_chunk(c):
            vp = vps.tile([128, D + 2], f32, tag="vp")
            xc = xT_sb[:, c * 128:(c + 1) * 128]
            nc.tensor.matmul(vp[:], xc, wv_sb[:])
            nc.vector.tensor_copy(vstage[:, c * 128:(c + 1) * 128],
                                  vp[:, :D])
            nc.vector.tensor_scalar_add(alpha_sb[:, c:c + 1],
                                        vp[:, D:D + 1], -CSHIFT)

        def emit_v_group(g0):
            lo_, hi_ = g0 * 128, (g0 + vgrp) * 128
            nc.gpsimd.dma_start(v16[:, lo_:hi_], vstage[:, lo_:hi_])

        ngx = (s + gxw - 1) // gxw
        emit_gx(0)
        for c in range(vgrp):
            emit_v_chunk(c)
        emit_v_group(0)
        for j in range(1, ngx):
            emit_gx(j)
        for g0 in range(vgrp, tch, vgrp):
            for c in range(g0, g0 + vgrp):
                emit_v_chunk(c)
            emit_v_group(g0)
        qkv_ctx.close()

        # schraudolph per-partition bias from alpha (single DVE op)
        nc.vector.tensor_scalar(alpha16[:], alpha_sb[:], SCH_A,
                                16256.5 + SCH_DELTA, ALU.mult, ALU.add)

        acc_ctx = ExitStack()
        scps = acc_ctx.enter_context(tc.tile_pool(name="scps", bufs=2,
                                                  space="PSUM"))
        accps = acc_ctx.enter_context(tc.tile_pool(name="accps", bufs=1,
                                                   space="PSUM"))
        denps = acc_ctx.enter_context(tc.tile_pool(name="denps", bufs=1,
                                                   space="PSUM"))

        def emit_scores(p, c):
            sc = scps.tile([128, sw], f32, tag="sc")
            gxc = gx_sb[:, c * 128:(c + 1) * 128]
            for j in range(jn):
                nc.tensor.matmul(sc[:, j * nw:(j + 1) * nw], gxc,
                                 xT_sb[:, p * sw + j * nw:
                                       p * sw + (j + 1) * nw])
            return sc

        def emit_exp(p, c, sc):
            """exp(SCALE*sc + alpha[c] - C) -> bf16 chunk tile."""
            et = exp_pool.tile([128, sw], bf16, name="et", tag="et")
            if sched[(p, c)] == "act":
                nc.scalar.activation(et[:], sc[:], FT.Exp,
                                     bias=alpha_sb[:, c:c + 1], scale=SCALE)
            else:
                # one DVE op straight into the bf16 bit pattern
                nc.vector.tensor_scalar(et[:].bitcast(i16), sc[:],
                                        SCALE * SCH_A, alpha16[:, c:c + 1],
                                        ALU.mult, ALU.add)
            return et

        # ---- attention passes
        for p in range(n_pass):
            acc_o = accps.tile([128, sw], f32, tag="acco")
            den_ps = denps.tile([128, sw], f32, tag="den")
            ets = {}
            qtiles = {}

            def emit_av(p, c, first, last):
                et = ets.pop((p, c))
                vc = v16[:, c * 128:(c + 1) * 128]
                for j in range(jn):
                    nc.tensor.matmul(acc_o[:, j * nw:(j + 1) * nw], vc,
                                     et[:, j * nw:(j + 1) * nw],
                                     start=first, stop=last)
                # quad-sum the exp chunks on DVE (bf16 4x mode); one
                # replicated ones-matmul per quad does the partition
                # reduction at 1/qd of the AV volume
                q = c // qd
                if c % qd == 0:
                    qtiles[q] = qs_pool.tile([128, sw], bf16, name="qt",
                                             tag="qt")
                    nc.vector.tensor_copy(qtiles[q][:], et[:])
                else:
                    nc.vector.tensor_add(qtiles[q][:], qtiles[q][:], et[:])
                if c % qd == qd - 1:
                    qt = qtiles.pop(q)
                    for j in range(jn):
                        nc.tensor.matmul(den_ps[:, j * nw:(j + 1) * nw],
                                         ones16[:],
                                         qt[:, j * nw:(j + 1) * nw],
                                         start=(q == 0),
                                         stop=(q == tch // qd - 1))

            # AV(c) emitted two chunks late so the PE always has scores
            # work while the exp/offload chains land
            for c in range(tch):
                sc = emit_scores(p, c)
                ets[(p, c)] = emit_exp(p, c, sc)
                if c >= 2:
                    emit_av(p, c - 2, first=(c == 2), last=False)
            emit_av(p, tch - 2, first=(tch == 2), last=False)
            emit_av(p, tch - 1, first=False, last=True)

            # stage num/den to SBUF (DVE), DMA out; the last pass drains
            # in quarters so the output DMA starts as early as possible
            num_sb = stage.tile([128, sw], f32, tag="num")
            den_sb = stage.tile([1, sw], f32, tag="densb")
            nq = 4 if p == n_pass - 1 else 1
            qw = sw // nq
            nc.vector.tensor_copy(den_sb[:], den_ps[0:1, :])
            nc.sync.dma_start(den_d[:, p * sw:(p + 1) * sw], den_sb[:])
            for qi in range(nq):
                nc.vector.tensor_copy(num_sb[:, qi * qw:(qi + 1) * qw],
                                      acc_o[:, qi * qw:(qi + 1) * qw])
                qengs[qi % 3].dma_start(
                    num_d[:, p * sw + qi * qw:p * sw + (qi + 1) * qw],
                    num_sb[:, qi * qw:(qi + 1) * qw])
        acc_ctx.close()
    nc.compile()
    return nc


def make_in_maps(x, Wq, bq, Wk, Wv, s=S, sq=SQ, n_cores=N_CORES):
    """Per-core inputs. Core c -> batch c//per_b, query half c%per_b via
    column rotation of x^T."""
    x = np.asarray(x, np.float64)
    nb = x.shape[0]
    per_b = n_cores // nb
    Wq = np.asarray(Wq, np.float64)
    Wk = np.asarray(Wk, np.float64)
    Wv = np.asarray(Wv, np.float64)
    bq = np.asarray(bq, np.float64)
    gT = (Wk.T @ Wq).astype(np.float32)               # [d', d]
    wtl = (SCALE * (Wk.T @ bq)).reshape(D, 1)
    wv_aug = np.concatenate([Wv.T, wtl, wtl], axis=1).astype(np.float32)
    maps = []
    for c in range(n_cores):
        b, h = c // per_b, c % per_b
        xt = np.ascontiguousarray(x[b].T.astype(np.float32))
        if h:
            xt = np.ascontiguousarray(
                np.concatenate([xt[:, h * sq:], xt[:, :h * sq]], axis=1))
        maps.append({"xT": xt, "gT": gT, "wvT": wv_aug})
    return maps


_NC_CACHE = {}


def _get_nc():
    if "nc" not in _NC_CACHE:
        _NC_CACHE["nc"] = build_attention_bass()
    return _NC_CACHE["nc"]


def postprocess(results, bv, x_shape=(B, S, D), n_cores=N_CORES, sq=SQ):
    """results[c] = {num: [D, sq], den: [1, sq]} -> full [B, S*D] output."""
    nb = x_shape[0]
    per_b = n_cores // nb
    bv = np.asarray(bv, np.float64).reshape(1, D)
    out = np.empty((nb, x_shape[1] * D), np.float32)
    for c in range(n_cores):
        b, h = c // per_b, c % per_b
        num = np.asarray(results[c]["num"], np.float64)   # [D, sq]
        den = np.asarray(results[c]["den"], np.float64)   # [1, sq]
        o = (num / den).T + bv                            # [sq, D]
        out[b, h * sq * D:(h + 1) * sq * D] = o.astype(np.float32).reshape(-1)
    return out


def run_on_hw(inputs, trace=False, **kw):
    from concourse.bass_utils import run_bass_kernel_spmd
    nc = _get_nc()
    maps = make_in_maps(inputs["x"], inputs["Wq"], inputs["bq"],
                        inputs["Wk"], inputs["Wv"])
    res = run_bass_kernel_spmd(nc, maps, core_ids=list(range(N_CORES)),
                               trace=trace, **kw)
    out = postprocess(res.results, inputs["bv"],
                      x_shape=np.asarray(inputs["x"]).shape)
    return out, res


def kernel(**inputs):
    out, _ = run_on_hw(inputs, trace=False)
    return out


# revision 32
# speedup vs baseline: 1.0738x; 1.0738x over previous
"""Trainium2 Bass kernel for nn_AttentionLayer (B=4, S=4096, D=128, fp32).

Sharding: batch (4) x query-half (2) across 8 NeuronCores; the query half is
realized by a host-side column ROTATION of x^T (keys are permutation
invariant under softmax+sum), so every core runs the identical SPMD program
with its queries at columns 0..sq-1.

Structure (per core):
  scores[t,s] = x_s^T (Wq^T Wk) x_t: host precomputes gT = Wk^T Wq (fp64);
    device: GX = gT^T @ x^T once, then scores chunks = GX-chunk^T @ x^T
    (f32r, 1 cyc/col). No Q/K projections or copies exist at all.
  bq folds into the exp bias alpha[t] = SCALE*bq.k_t (extra V-proj column);
    bk cancels in softmax; bv is applied on host; a global shift C in the
    exp cancels in softmax (numerics centering).
  exp -> bf16 tiles. A slice of chunks runs as int16-Schraudolph on DVE
    (one tensor_scalar into the bf16 bit pattern) concurrently with ACT
    exps so the PE is never exp-starved (p-state stays at 2.4 GHz).
  AV: per-chunk bf16 matmuls (V in bf16: quantization error ~0.4%).
  denominator: DVE quad-sums exp chunks (bf16 4x mode, 0.25 cyc/col),
    then one replicated ones-matmul per quad accumulates in PSUM - the
    partition reduction only the PE can do, at 1/4 the matmul volume.
  num/den ship to host; host does num/den + bv in fp64 (free).

Measured HW facts this design is built on: every matmul costs ~1 cycle
per OUTPUT column at 2.4GHz (fp8 DoubleRow included, so fp8 buys nothing
once V needs hi+lo); ACT exp [128,1024] = 1.07us; DVE tensor_scalar from
PSUM = 1.19us; GPSIMD cannot touch PSUM and its casts are ~3.5us (but it
can initiate casting DMAs); PE de-ramps to 1.2GHz whenever it idles.
"""

import sys

import numpy as np

for _p in ("/opt/trn_rl_repo", "/opt/pypackages"):
    if _p not in sys.path:
        sys.path.append(_p)

B, S, D = 4, 4096, 128
N_CORES = 8
SQ = S // 2            # queries per core
SCALE = 1.0 / float(np.sqrt(D))
CSHIFT = 1.5           # global exp shift: exp(y-C); cancels in softmax
# Schraudolph (bf16 bit pattern): i16 = y*184.6635 + 16256.5 + delta
SCH_A = 128.0 / float(np.log(2.0))
SCH_DELTA = -7.0       # centers the 2^frac linear-interp overestimate


def default_exp_sched(n_pass, tch, n_dve=0):
    """Per (pass, chunk) exp engine: 'act' or 'dve' (Schraudolph).
    n_dve: int or per-pass list = offloaded chunks per pass. Offloaded
    chunks are never adjacent (DVE serializes; ACT must run concurrently)
    and sit late in pass 0 (V-prep owns the DVE early on)."""
    if isinstance(n_dve, int):
        n_dve = [n_dve] * n_pass
    sched = {}
    npair = tch // 2
    for p in range(n_pass):
        nd = min(n_dve[p] if p < len(n_dve) else n_dve[-1], npair)
        if p == 0:
            gset = set(range(npair - 2, max(-1, npair - 2 - nd), -1))
        else:
            gset = set(range(1, min(npair, 1 + nd)))
        for c in range(tch):
            g = c // 2
            sched[(p, c)] = "dve" if (g in gset and c % 2 == 1) else "act"
    return sched


def build_attention_bass(s=S, sq=SQ, sw=1024, n_dve_exp=(2, 4)):
    """Single-core SPMD program. s: keys; sq: queries; sw: pass width."""
    import concourse.bass as bass
    import concourse.mybir as mybir
    import concourse.tile as tile
    from concourse import bacc
    from contextlib import ExitStack

    f32 = mybir.dt.float32
    f32r = mybir.dt.float32r
    bf16 = mybir.dt.bfloat16
    i16 = mybir.dt.int16
    FT = mybir.ActivationFunctionType
    ALU = mybir.AluOpType

    tch = s // 128          # key chunks (128 keys each)
    n_pass = sq // sw
    nw = min(512, sw)       # matmul N width (f32r needs >=256; ISA caps
                            # output at 512 cols / one PSUM bank)
    jn = sw // nw
    gxw = min(512, s)       # GX matmul chunk width
    qd = 4 if tch % 4 == 0 else 2   # chunks per denominator quad-sum
    sched = default_exp_sched(n_pass, tch, n_dve_exp)

    nc = bacc.Bacc("TRN2", target_bir_lowering=False, debug=False)

    xT = nc.dram_tensor("xT", [D, s], f32r, kind="ExternalInput").ap()
    gT = nc.dram_tensor("gT", [D, D], f32r, kind="ExternalInput").ap()
    wvT = nc.dram_tensor("wvT", [D, D + 2], f32r, kind="ExternalInput").ap()
    num_d = nc.dram_tensor("num", [D, sq], f32, kind="ExternalOutput").ap()
    den_d = nc.dram_tensor("den", [1, sq], f32, kind="ExternalOutput").ap()

    with tile.TileContext(nc) as tc, ExitStack() as ctx:
        const = ctx.enter_context(tc.tile_pool(name="const", bufs=1))
        big = ctx.enter_context(tc.tile_pool(name="big", bufs=1))
        exp_pool = ctx.enter_context(tc.tile_pool(name="exp", bufs=8))
        qs_pool = ctx.enter_context(tc.tile_pool(name="qs", bufs=2))
        stage = ctx.enter_context(tc.tile_pool(name="stage", bufs=2))

        gT_sb = const.tile([D, D], f32r, tag="gT")
        wv_sb = const.tile([D, D + 2], f32r, tag="wv")
        ones16 = const.tile([128, 128], bf16, tag="ones16")
        alpha_sb = const.tile([128, tch], f32, tag="alpha")    # alpha - C
        alpha16 = const.tile([128, tch], f32, tag="alpha16")   # schraudolph

        xT_sb = big.tile([D, s], f32r, tag="xT")
        gx_sb = big.tile([D, s], f32r, tag="gx")
        v16 = big.tile([128, s], bf16, tag="v16")    # V in [t, e] layout
        vstage = big.tile([128, s], f32, tag="vstage")

        # ---- input DMAs: many small transfers engage more of the 16 DMA
        # engines; initiation spread over three queue engines; ascending
        # order so early columns (first GX/V chunks, pass-0 queries) land
        # first.
        nc.sync.dma_start(gT_sb[:], gT)
        nc.sync.dma_start(wv_sb[:], wvT)
        xw = min(256, s)
        qengs = [nc.sync, nc.scalar, nc.gpsimd]
        for i, st in enumerate(range(0, s, xw)):
            w = min(xw, s - st)
            qengs[i % 3].dma_start(xT_sb[:, st:st + w], xT[:, st:st + w])
        nc.vector.memset(ones16[:], 1.0)

        # ---- phase A: GX projection + V/alpha; PSUM pools closed after
        qkv_ctx = ExitStack()
        gxps = qkv_ctx.enter_context(tc.tile_pool(name="gxps", bufs=3,
                                                  space="PSUM"))
        vps = qkv_ctx.enter_context(tc.tile_pool(name="vps", bufs=3,
                                                 space="PSUM"))

        def emit_gx(j):
            st, w = j * gxw, min(gxw, s - j * gxw)
            gp = gxps.tile([128, gxw], f32, tag="gx")
            nc.tensor.matmul(gp[:, :w], gT_sb[:], xT_sb[:, st:st + w])
            # ACT is idle before the first exp; DVE stays free for V-prep
            nc.scalar.copy(gx_sb[:, st:st + w], gp[:, :w])

        # V-prep: per chunk a PSUM->SBUF f32 stage copy + alpha extract on
        # DVE; the f32->bf16 cast is a bulk casting DMA per group of 8.
        vgrp = min(8, tch)

        def emit_v_chunk(c):
            vp = vps.tile([128, D + 2], f32, tag="vp")
            xc = xT_sb[:, c * 128:(c + 1) * 128]
            nc.tensor.matmul(vp[:], xc, wv_sb[:])
            nc.vector.tensor_copy(vstage[:, c * 128:(c + 1) * 128],
                                  vp[:, :D])
            nc.vector.tensor_scalar_add(alpha_sb[:, c:c + 1],
                                        vp[:, D:D + 1], -CSHIFT)

        def emit_v_group(g0):
            lo_, hi_ = g0 * 128, (g0 + vgrp) * 128
            nc.gpsimd.dma_start(v16[:, lo_:hi_], vstage[:, lo_:hi_])

        ngx = (s + gxw - 1) // gxw
        emit_gx(0)
        for c in range(vgrp):
            emit_v_chunk(c)
        emit_v_group(0)
        for j in range(1, ngx):
            emit_gx(j)
        for g0 in range(vgrp, tch, vgrp):
            for c in range(g0, g0 + vgrp):
                emit_v_chunk(c)
            emit_v_group(g0)
        qkv_ctx.close()

        # schraudolph per-partition bias from alpha (single DVE op)
        nc.vector.tensor_scalar(alpha16[:], alpha_sb[:], SCH_A,
                                16256.5 + SCH_DELTA, ALU.mult, ALU.add)

        acc_ctx = ExitStack()
        scps = acc_ctx.enter_context(tc.tile_pool(name="scps", bufs=2,
                                                  space="PSUM"))
        accps = acc_ctx.enter_context(tc.tile_pool(name="accps", bufs=1,
                                                   space="PSUM"))
        denps = acc_ctx.enter_context(tc.tile_pool(name="denps", bufs=1,
                                                   space="PSUM"))

        def emit_scores(p, c):
            sc = scps.tile([128, sw], f32, tag="sc")
            gxc = gx_sb[:, c * 128:(c + 1) * 128]
            for j in range(jn):
                nc.tensor.matmul(sc[:, j * nw:(j + 1) * nw], gxc,
                                 xT_sb[:, p * sw + j * nw:
                                       p * sw + (j + 1) * nw])
            return sc

        def emit_exp(p, c, sc):
            """exp(SCALE*sc + alpha[c] - C) -> bf16 chunk tile."""
            et = exp_pool.tile([128, sw], bf16, name="et", tag="et")
            if sched[(p, c)] == "act":
                nc.scalar.activation(et[:], sc[:], FT.Exp,
                                     bias=alpha_sb[:, c:c + 1], scale=SCALE)
            else:
                # one DVE op straight into the bf16 bit pattern
                nc.vector.tensor_scalar(et[:].bitcast(i16), sc[:],
                                        SCALE * SCH_A, alpha16[:, c:c + 1],
                                        ALU.mult, ALU.add)
            return et

        # ---- attention passes
        for p in range(n_pass):
            acc_o = accps.tile([128, sw], f32, tag="acco")
            den_ps = denps.tile([128, sw], f32, tag="den")
            ets = {}
            qtiles = {}

            def emit_av(p, c, first, last):
                et = ets.pop((p, c))
                vc = v16[:, c * 128:(c + 1) * 128]
                for j in range(jn):
                    nc.tensor.matmul(acc_o[:, j * nw:(j + 1) * nw], vc,
                                     et[:, j * nw:(j + 1) * nw],
                                     start=first, stop=last)
                # quad-sum the exp chunks (bf16, in place into the quad's
                # first chunk tile; one independent add per quad goes to
                # the otherwise-idle Pool engine); a replicated
                # ones-matmul per quad then does the partition reduction
                # at 1/qd of the AV volume
                q = c // qd
                if c % qd == 0:
                    qtiles[q] = [et]
                else:
                    qtiles[q].append(et)
                if c % qd == qd - 1:
                    grp = qtiles.pop(q)
                    q0 = grp[0]
                    if len(grp) == 4:
                        tmp = qs_pool.tile([128, sw], bf16, name="qt",
                                           tag="qt")
                        nc.gpsimd.tensor_add(tmp[:], grp[2][:], grp[3][:])
                        nc.vector.tensor_add(q0[:], q0[:], grp[1][:])
                        nc.vector.tensor_add(q0[:], q0[:], tmp[:])
                    else:
                        for other in grp[1:]:
                            nc.vector.tensor_add(q0[:], q0[:], other[:])
                    for j in range(jn):
                        nc.tensor.matmul(den_ps[:, j * nw:(j + 1) * nw],
                                         ones16[:],
                                         q0[:, j * nw:(j + 1) * nw],
                                         start=(q == 0),
                                         stop=(q == tch // qd - 1))

            # AV(c) emitted two chunks late so the PE always has scores
            # work while the exp/offload chains land
            for c in range(tch):
                sc = emit_scores(p, c)
                ets[(p, c)] = emit_exp(p, c, sc)
                if c >= 2:
                    emit_av(p, c - 2, first=(c == 2), last=False)
            emit_av(p, tch - 2, first=(tch == 2), last=False)
            emit_av(p, tch - 1, first=False, last=True)

            # stage num/den to SBUF (DVE), DMA out; the last pass drains
            # in quarters so the output DMA starts as early as possible
            num_sb = stage.tile([128, sw], f32, tag="num")
            den_sb = stage.tile([1, sw], f32, tag="densb")
            nq = 4 if p == n_pass - 1 else 1
            qw = sw // nq
            nc.vector.tensor_copy(den_sb[:], den_ps[0:1, :])
            nc.sync.dma_start(den_d[:, p * sw:(p + 1) * sw], den_sb[:])
            for qi in range(nq):
                nc.vector.tensor_copy(num_sb[:, qi * qw:(qi + 1) * qw],
                                      acc_o[:, qi * qw:(qi + 1) * qw])
                qengs[qi % 3].dma_start(
                    num_d[:, p * sw + qi * qw:p * sw + (qi + 1) * qw],
                    num_sb[:, qi * qw:(qi + 1) * qw])
        acc_ctx.close()
    nc.compile()
    return nc


def make_in_maps(x, Wq, bq, Wk, Wv, s=S, sq=SQ, n_cores=N_CORES):
    """Per-core inputs. Core c -> batch c//per_b, query half c%per_b via
    column rotation of x^T."""
    x = np.asarray(x, np.float64)
    nb = x.shape[0]
    per_b = n_cores // nb
    Wq = np.asarray(Wq, np.float64)
    Wk = np.asarray(Wk, np.float64)
    Wv = np.asarray(Wv, np.float64)
    bq = np.asarray(bq, np.float64)
    gT = (Wk.T @ Wq).astype(np.float32)               # [d', d]
    wtl = (SCALE * (Wk.T @ bq)).reshape(D, 1)
    wv_aug = np.concatenate([Wv.T, wtl, wtl], axis=1).astype(np.float32)
    maps = []
    for c in range(n_cores):
        b, h = c // per_b, c % per_b
        xt = np.ascontiguousarray(x[b].T.astype(np.float32))
        if h:
            xt = np.ascontiguousarray(
                np.concatenate([xt[:, h * sq:], xt[:, :h * sq]], axis=1))
        maps.append({"xT": xt, "gT": gT, "wvT": wv_aug})
    return maps


_NC_CACHE = {}


def _get_nc():
    if "nc" not in _NC_CACHE:
        _NC_CACHE["nc"] = build_attention_bass()
    return _NC_CACHE["nc"]


def postprocess(results, bv, x_shape=(B, S, D), n_cores=N_CORES, sq=SQ):
    """results[c] = {num: [D, sq], den: [1, sq]} -> full [B, S*D] output."""
    nb = x_shape[0]
    per_b = n_cores // nb
    bv = np.asarray(bv, np.float64).reshape(1, D)
    out = np.empty((nb, x_shape[1] * D), np.float32)
    for c in range(n_cores):
        b, h = c // per_b, c % per_b
        num = np.asarray(results[c]["num"], np.float64)   # [D, sq]
        den = np.asarray(results[c]["den"], np.float64)   # [1, sq]
        o = (num / den).T + bv                            # [sq, D]
        out[b, h * sq * D:(h + 1) * sq * D] = o.astype(np.float32).reshape(-1)
    return out


def run_on_hw(inputs, trace=False, **kw):
    from concourse.bass_utils import run_bass_kernel_spmd
    nc = _get_nc()
    maps = make_in_maps(inputs["x"], inputs["Wq"], inputs["bq"],
                        inputs["Wk"], inputs["Wv"])
    res = run_bass_kernel_spmd(nc, maps, core_ids=list(range(N_CORES)),
                               trace=trace, **kw)
    out = postprocess(res.results, inputs["bv"],
                      x_shape=np.asarray(inputs["x"]).shape)
    return out, res


def kernel(**inputs):
    out, _ = run_on_hw(inputs, trace=False)
    return out


# revision 39
# speedup vs baseline: 1.1854x; 1.1039x over previous
"""Trainium2 Bass kernel for nn_AttentionLayer (B=4, S=4096, D=128, fp32).

Sharding: batch (4) x query-half (2) across 8 NeuronCores; the query half is
realized by a host-side column ROTATION of x^T (keys are permutation
invariant under softmax+sum), so every core runs the identical SPMD program
with its queries at columns 0..sq-1.

Structure (per core):
  scores[t,s] = x_s^T (Wq^T Wk) x_t: host precomputes gT = Wk^T Wq (fp64);
    device: GX = gT^T @ x^T once, then scores chunks = GX-chunk^T @ x^T
    (f32r, 1 cyc/col). No Q/K projections or copies exist at all.
  bq folds into the exp bias alpha[t] = SCALE*bq.k_t (extra V-proj column);
    bk cancels in softmax; bv is applied on host; a global shift C in the
    exp cancels in softmax (numerics centering).
  exp -> bf16 tiles. A slice of chunks runs as int16-Schraudolph on DVE
    (one tensor_scalar into the bf16 bit pattern) concurrently with ACT
    exps so the PE is never exp-starved (p-state stays at 2.4 GHz).
  AV: per-chunk bf16 matmuls (V in bf16: quantization error ~0.4%).
  denominator: DVE quad-sums exp chunks (bf16 4x mode, 0.25 cyc/col),
    then one replicated ones-matmul per quad accumulates in PSUM - the
    partition reduction only the PE can do, at 1/4 the matmul volume.
  num/den ship to host; host does num/den + bv in fp64 (free).

Measured HW facts this design is built on: every matmul costs ~1 cycle
per OUTPUT column at 2.4GHz (fp8 DoubleRow included, so fp8 buys nothing
once V needs hi+lo); ACT exp [128,1024] = 1.07us; DVE tensor_scalar from
PSUM = 1.19us; GPSIMD cannot touch PSUM and its casts are ~3.5us (but it
can initiate casting DMAs); PE de-ramps to 1.2GHz whenever it idles.
"""

import sys

import numpy as np

for _p in ("/opt/trn_rl_repo", "/opt/pypackages"):
    if _p not in sys.path:
        sys.path.append(_p)

B, S, D = 4, 4096, 128
N_CORES = 8
SQ = S // 2            # queries per core
SCALE = 1.0 / float(np.sqrt(D))
CSHIFT = 1.5           # global exp shift: exp(y-C); cancels in softmax
# Schraudolph (bf16 bit pattern): i16 = y*184.6635 + 16256.5 + delta
SCH_A = 128.0 / float(np.log(2.0))
SCH_DELTA = -7.0       # centers the 2^frac linear-interp overestimate


def default_exp_sched(n_pass, tch, n_dve=0):
    """Per (pass, chunk) exp engine: 'act' or 'dve' (Schraudolph).
    n_dve: int or per-pass list = offloaded chunks per pass. Offloaded
    chunks are never adjacent (DVE serializes; ACT must run concurrently)
    and sit late in pass 0 (V-prep owns the DVE early on)."""
    if isinstance(n_dve, int):
        n_dve = [n_dve] * n_pass
    sched = {}
    npair = tch // 2
    for p in range(n_pass):
        nd = min(n_dve[p] if p < len(n_dve) else n_dve[-1], npair)
        if p == 0:
            gset = set(range(npair - 2, max(-1, npair - 2 - nd), -1))
        else:
            gset = set(range(1, min(npair, 1 + nd)))
        for c in range(tch):
            g = c // 2
            sched[(p, c)] = "dve" if (g in gset and c % 2 == 1) else "act"
    return sched


def build_attention_bass(s=S, sq=SQ, sw=1024, n_dve_exp=(3, 8)):
    """Single-core SPMD program. s: keys; sq: queries; sw: pass width."""
    import concourse.bass as bass
    import concourse.mybir as mybir
    import concourse.tile as tile
    from concourse import bacc
    from contextlib import ExitStack

    f32 = mybir.dt.float32
    f32r = mybir.dt.float32r
    bf16 = mybir.dt.bfloat16
    i16 = mybir.dt.int16
    FT = mybir.ActivationFunctionType
    ALU = mybir.AluOpType

    tch = s // 128          # key chunks (128 keys each)
    n_pass = sq // sw
    nw = min(512, sw)       # matmul N width (f32r needs >=256; ISA caps
                            # output at 512 cols / one PSUM bank)
    jn = sw // nw
    gxw = min(512, s)       # GX matmul chunk width
    qd = 4 if tch % 4 == 0 else 2   # chunks per denominator quad-sum
    sched = default_exp_sched(n_pass, tch, n_dve_exp)

    nc = bacc.Bacc("TRN2", target_bir_lowering=False, debug=False)

    # x and Wv ship as bf16: halves the input-DMA wall; scores/GX keep an
    # f32r stationary with a bf16 moving operand (1 cyc/col either way)
    xT = nc.dram_tensor("xT", [D, s], bf16, kind="ExternalInput").ap()
    gT = nc.dram_tensor("gT", [D, D], bf16, kind="ExternalInput").ap()
    wvT = nc.dram_tensor("wvT", [D, D + 2], bf16, kind="ExternalInput").ap()
    num_d = nc.dram_tensor("num", [D, sq], f32, kind="ExternalOutput").ap()
    den_d = nc.dram_tensor("den", [1, sq], f32, kind="ExternalOutput").ap()

    with tile.TileContext(nc) as tc, ExitStack() as ctx:
        const = ctx.enter_context(tc.tile_pool(name="const", bufs=1))
        big = ctx.enter_context(tc.tile_pool(name="big", bufs=1))
        exp_pool = ctx.enter_context(tc.tile_pool(name="exp", bufs=8))
        qs_pool = ctx.enter_context(tc.tile_pool(name="qs", bufs=2))
        stage = ctx.enter_context(tc.tile_pool(name="stage", bufs=2))

        gT_sb = const.tile([D, D], bf16, tag="gT")
        wv_sb = const.tile([D, D + 2], bf16, tag="wv")
        ones16 = const.tile([128, 128], bf16, tag="ones16")
        alpha_sb = const.tile([128, tch], f32, tag="alpha")    # alpha - C
        alpha16 = const.tile([128, tch], f32, tag="alpha16")   # schraudolph

        xT_sb = big.tile([D, s], bf16, tag="xT")
        gx_sb = big.tile([D, s], bf16, tag="gx")
        v16 = big.tile([128, s], bf16, tag="v16")    # V in [t, e] layout
        vstage = big.tile([128, s], f32, tag="vstage")

        # ---- input DMAs: many small transfers engage more of the 16 DMA
        # engines; initiation spread over three queue engines; ascending
        # order so early columns (first GX/V chunks, pass-0 queries) land
        # first.
        nc.sync.dma_start(gT_sb[:], gT)
        nc.sync.dma_start(wv_sb[:], wvT)
        xw = min(256, s)
        qengs = [nc.sync, nc.scalar, nc.gpsimd]
        for i, st in enumerate(range(0, s, xw)):
            w = min(xw, s - st)
            qengs[i % 3].dma_start(xT_sb[:, st:st + w], xT[:, st:st + w])
        nc.vector.memset(ones16[:], 1.0)

        # ---- phase A: GX projection + V/alpha; PSUM pools closed after
        qkv_ctx = ExitStack()
        gxps = qkv_ctx.enter_context(tc.tile_pool(name="gxps", bufs=3,
                                                  space="PSUM"))
        vps = qkv_ctx.enter_context(tc.tile_pool(name="vps", bufs=3,
                                                 space="PSUM"))

        def emit_gx(j):
            st, w = j * gxw, min(gxw, s - j * gxw)
            gp = gxps.tile([128, gxw], f32, tag="gx")
            nc.tensor.matmul(gp[:, :w], gT_sb[:], xT_sb[:, st:st + w])
            # ACT is idle before the first exp; DVE stays free for V-prep
            nc.scalar.copy(gx_sb[:, st:st + w], gp[:, :w])

        # V-prep: per chunk a PSUM->SBUF f32 stage copy + alpha extract on
        # DVE; the f32->bf16 cast is a bulk casting DMA per group of 8.
        vgrp = min(8, tch)

        def emit_v_chunk(c):
            vp = vps.tile([128, D + 2], f32, tag="vp")
            xc = xT_sb[:, c * 128:(c + 1) * 128]
            nc.tensor.matmul(vp[:], xc, wv_sb[:])
            nc.vector.tensor_copy(vstage[:, c * 128:(c + 1) * 128],
                                  vp[:, :D])
            nc.vector.tensor_scalar_add(alpha_sb[:, c:c + 1],
                                        vp[:, D:D + 1], -CSHIFT)

        def emit_v_group(g0):
            lo_, hi_ = g0 * 128, (g0 + vgrp) * 128
            nc.gpsimd.dma_start(v16[:, lo_:hi_], vstage[:, lo_:hi_])

        ngx = (s + gxw - 1) // gxw
        emit_gx(0)
        for c in range(vgrp):
            emit_v_chunk(c)
        emit_v_group(0)
        for j in range(1, ngx):
            emit_gx(j)
        for g0 in range(vgrp, tch, vgrp):
            for c in range(g0, g0 + vgrp):
                emit_v_chunk(c)
            emit_v_group(g0)
        qkv_ctx.close()

        # schraudolph per-partition bias from alpha (single DVE op)
        nc.vector.tensor_scalar(alpha16[:], alpha_sb[:], SCH_A,
                                16256.5 + SCH_DELTA, ALU.mult, ALU.add)

        acc_ctx = ExitStack()
        scps = acc_ctx.enter_context(tc.tile_pool(name="scps", bufs=2,
                                                  space="PSUM"))
        accps = acc_ctx.enter_context(tc.tile_pool(name="accps", bufs=1,
                                                   space="PSUM"))
        denps = acc_ctx.enter_context(tc.tile_pool(name="denps", bufs=1,
                                                   space="PSUM"))

        def emit_scores(p, c):
            sc = scps.tile([128, sw], f32, tag="sc")
            gxc = gx_sb[:, c * 128:(c + 1) * 128]
            for j in range(jn):
                nc.tensor.matmul(sc[:, j * nw:(j + 1) * nw], gxc,
                                 xT_sb[:, p * sw + j * nw:
                                       p * sw + (j + 1) * nw])
            return sc

        def emit_exp(p, c, sc):
            """exp(SCALE*sc + alpha[c] - C) -> bf16 chunk tile."""
            et = exp_pool.tile([128, sw], bf16, name="et", tag="et")
            if sched[(p, c)] == "act":
                nc.scalar.activation(et[:], sc[:], FT.Exp,
                                     bias=alpha_sb[:, c:c + 1], scale=SCALE)
            else:
                # one DVE op straight into the bf16 bit pattern
                nc.vector.tensor_scalar(et[:].bitcast(i16), sc[:],
                                        SCALE * SCH_A, alpha16[:, c:c + 1],
                                        ALU.mult, ALU.add)
            return et

        # ---- attention passes
        for p in range(n_pass):
            acc_o = accps.tile([128, sw], f32, tag="acco")
            den_ps = denps.tile([128, sw], f32, tag="den")
            ets = {}
            qtiles = {}
            denq = []

            def emit_av(p, c, first, last):
                et = ets.pop((p, c))
                vc = v16[:, c * 128:(c + 1) * 128]
                for j in range(jn):
                    nc.tensor.matmul(acc_o[:, j * nw:(j + 1) * nw], vc,
                                     et[:, j * nw:(j + 1) * nw],
                                     start=first, stop=last)
                # quad-sum the exp chunks (bf16, in place into the quad's
                # first chunk tile; one independent add per quad on the
                # otherwise-idle Pool engine). The replicated ones-matmul
                # (partition reduction at 1/qd of AV volume) is deferred
                # via denq so the PE never waits the add chain.
                q = c // qd
                qtiles.setdefault(q, []).append(et)
                if c % qd == qd - 1:
                    grp = qtiles.pop(q)
                    q0 = grp[0]
                    if len(grp) == 4:
                        tmp = qs_pool.tile([128, sw], bf16, name="qt",
                                           tag="qt")
                        nc.gpsimd.tensor_add(tmp[:], grp[2][:], grp[3][:])
                        nc.vector.tensor_add(q0[:], q0[:], grp[1][:])
                        nc.vector.tensor_add(q0[:], q0[:], tmp[:])
                    else:
                        for other in grp[1:]:
                            nc.vector.tensor_add(q0[:], q0[:], other[:])
                    denq.append((q, q0))

            def emit_den(last_pass_tail=False):
                q, q0 = denq.pop(0)
                for j in range(jn):
                    nc.tensor.matmul(den_ps[:, j * nw:(j + 1) * nw],
                                     ones16[:],
                                     q0[:, j * nw:(j + 1) * nw],
                                     start=(q == 0),
                                     stop=(q == tch // qd - 1))

            # AV(c) emitted two chunks late so the PE always has scores
            # work while the exp/offload chains land; den matmuls lag a
            # further two chunks behind their quad's completion
            for c in range(tch):
                sc = emit_scores(p, c)
                ets[(p, c)] = emit_exp(p, c, sc)
                if c >= 2:
                    emit_av(p, c - 2, first=(c == 2), last=False)
                if denq and (c - 2) % qd == 1:
                    emit_den()
            emit_av(p, tch - 2, first=(tch == 2), last=False)
            emit_av(p, tch - 1, first=False, last=True)
            while denq:
                emit_den()

            # stage num/den to SBUF (DVE), DMA out; the last pass drains
            # in quarters so the output DMA starts as early as possible
            num_sb = stage.tile([128, sw], f32, tag="num")
            den_sb = stage.tile([1, sw], f32, tag="densb")
            nq = 4 if p == n_pass - 1 else 1
            qw = sw // nq
            nc.vector.tensor_copy(den_sb[:], den_ps[0:1, :])
            nc.sync.dma_start(den_d[:, p * sw:(p + 1) * sw], den_sb[:])
            for qi in range(nq):
                nc.vector.tensor_copy(num_sb[:, qi * qw:(qi + 1) * qw],
                                      acc_o[:, qi * qw:(qi + 1) * qw])
                qengs[qi % 3].dma_start(
                    num_d[:, p * sw + qi * qw:p * sw + (qi + 1) * qw],
                    num_sb[:, qi * qw:(qi + 1) * qw])
        acc_ctx.close()
    nc.compile()
    return nc


def make_in_maps(x, Wq, bq, Wk, Wv, s=S, sq=SQ, n_cores=N_CORES):
    """Per-core inputs. Core c -> batch c//per_b, query half c%per_b via
    column rotation of x^T."""
    x = np.asarray(x, np.float64)
    nb = x.shape[0]
    per_b = n_cores // nb
    Wq = np.asarray(Wq, np.float64)
    Wk = np.asarray(Wk, np.float64)
    Wv = np.asarray(Wv, np.float64)
    bq = np.asarray(bq, np.float64)
    gT = (Wk.T @ Wq)                                  # [d', d]
    wtl = (SCALE * (Wk.T @ bq)).reshape(D, 1)
    wv_aug = np.concatenate([Wv.T, wtl, wtl], axis=1).astype(np.float32)
    import ml_dtypes
    wv16 = wv_aug.astype(ml_dtypes.bfloat16)
    gT16 = np.ascontiguousarray(gT.astype(ml_dtypes.bfloat16))
    maps = []
    for c in range(n_cores):
        b, h = c // per_b, c % per_b
        xt = x[b].T
        if h:
            xt = np.concatenate([xt[:, h * sq:], xt[:, :h * sq]], axis=1)
        xt16 = np.ascontiguousarray(xt.astype(ml_dtypes.bfloat16))
        maps.append({"xT": xt16, "gT": gT16, "wvT": wv16})
    return maps


_NC_CACHE = {}


def _get_nc():
    if "nc" not in _NC_CACHE:
        _NC_CACHE["nc"] = build_attention_bass()
    return _NC_CACHE["nc"]


def postprocess(results, bv, x_shape=(B, S, D), n_cores=N_CORES, sq=SQ):
    """results[c] = {num: [D, sq], den: [1, sq]} -> full [B, S*D] output."""
    nb = x_shape[0]
    per_b = n_cores // nb
    bv = np.asarray(bv, np.float64).reshape(1, D)
    out = np.empty((nb, x_shape[1] * D), np.float32)
    for c in range(n_cores):
        b, h = c // per_b, c % per_b
        num = np.asarray(results[c]["num"], np.float64)   # [D, sq]
        den = np.asarray(results[c]["den"], np.float64)   # [1, sq]
        o = (num / den).T + bv                            # [sq, D]
        out[b, h * sq * D:(h + 1) * sq * D] = o.astype(np.float32).reshape(-1)
    return out


def run_on_hw(inputs, trace=False, **kw):
    from concourse.bass_utils import run_bass_kernel_spmd
    nc = _get_nc()
    maps = make_in_maps(inputs["x"], inputs["Wq"], inputs["bq"],
                        inputs["Wk"], inputs["Wv"])
    res = run_bass_kernel_spmd(nc, maps, core_ids=list(range(N_CORES)),
                               trace=trace, **kw)
    out = postprocess(res.results, inputs["bv"],
                      x_shape=np.asarray(inputs["x"]).shape)
    return out, res


def kernel(**inputs):
    out, _ = run_on_hw(inputs, trace=False)
    return out
